# revision 27
# baseline (speedup 1.0000x reference)
"""LTPE kernel for Trainium2: RGB->gray, 8-neighbor weighted diff encoding,
instance norm, replicated to 3 channels.  Data-parallel over batch: one
sample per NeuronCore (8 cores).

Math: with g = 0.3 x0 + 0.59 x1 + 0.11 x2 and weights w_j = 2^j/255 at the
8 neighbor offsets, the reference output before the norm is
0.055*z + 0.5 where z = G - sum_j w_j * shift_j(G), G = g/0.11.
Instance norm is affine-invariant, so out = (z - mean_z) * rsqrt(var_z + EPS_EFF)
with EPS_EFF = 1e-5 / 0.055^2.

The 9-tap stencil is computed on the TensorEngine: for each column shift
dj in {-1,0,+1} a tridiagonal [128,126] lhsT applies all three row taps in
one matmul (PSUM-accumulated), on halo-free 126-row blocks.

v2: all input DMAs are issued up-front, alternating between the two HWDGE
rings (sync + scalar), so many transfers are in flight concurrently from
t=0 — a single in-flight HBM load drains at ~26 GB/s (one SDMA engine),
saturating ~358 GB/s needs 10+ concurrent transfers.  The normalize pass
is split between DVE and ACT and stores alternate rings too.
"""

import sys

sys.path.insert(0, "/opt/trn_rl_repo")

import numpy as np

import concourse.bass as bass
import concourse.mybir as mybir
import concourse.tile as tile
from concourse.vector_clock import ScopedClock

B, C, H, W = 8, 3, 1024, 1024
NCORES = 8
Q = 126              # output rows per block
NBLK = 9             # 8 full blocks + 1 tail block of 16 rows
EPS_EFF = 1e-5 / (0.5 * 0.11) ** 2
USE_F32R = True      # full-rate fp32 matmul mode (validated vs fp32 on hw)

# neighbor offsets (di, dj) -> bit j;  kernel j weight = 2^j/255
OFFS = {(0, -1): 0, (1, -1): 1, (1, 0): 2, (1, 1): 3,
        (0, 1): 4, (-1, 1): 5, (-1, 0): 6, (-1, -1): 7}


def _tap(di, dj):
    v = 1.0 if (di == 0 and dj == 0) else 0.0
    if (di, dj) in OFFS:
        v -= 2.0 ** OFFS[(di, dj)] / 255.0
    return v


def _build_weights():
    # Interior blocks (cols 0-2): input partitions hold image rows r0-1+k,
    # so w[k, dji, m] = tap(k - m - 1, dj), nonzero at k = m, m+1, m+2.
    # Block 0 (cols 3-5): input starts at image row 0 (no row above), so
    # w[k, 3+dji, m] = tap(k - m, dj), nonzero at k = m-1, m, m+1.
    # Tail block (cols 6-8): input partitions hold rows 896+k (a full
    # 128-row window ending at the image bottom), output rows 1008+m for
    # m = 0..15, so w[k, 6+dji, m] = tap(k - 112 - m, dj), nonzero at
    # k = 111+m, 112+m, 113+m (k = 128 falls off the bottom = zero pad).
    w = np.zeros((128, 9, Q), np.float32)
    for dji, dj in enumerate((-1, 0, 1)):
        for m in range(Q):
            for k in (m, m + 1, m + 2):
                if k < 128:
                    w[k, dji, m] = _tap(k - m - 1, dj)
            for k in (m - 1, m, m + 1):
                if 0 <= k < 128:
                    w[k, 3 + dji, m] = _tap(k - m, dj)
        for m in range(H - Q * (NBLK - 1)):
            for k in (111 + m, 112 + m, 113 + m):
                if 0 <= k < 128:
                    w[k, 6 + dji, m] = _tap(k - 112 - m, dj)
    return w


def _patched_drain_and_barrier(self, tick_clock, wait_clock):
    # walrus rejects >1-2 sync waits on the kernel-tail Drain (CTRL
    # NO_STRUCT codegen); spread the global-clock waits one-per-nop.
    nc = self.nc
    carrier = nc.sync.nop()
    wait_clock.add_sem_waits(carrier.ins, ScopedClock({None: tick_clock.global_clock}))
    waits = list(carrier.ins.sync_info.on_wait or [])
    if len(waits) > 1:
        carrier.ins.sync_info.on_wait = waits[:1]
        for wt in waits[1:]:
            n = nc.sync.nop()
            n.ins.sync_info = mybir.SyncInfo(on_wait=[wt], on_update=[])
    nc.sync.drain()
    nc.all_engine_barrier()
    assert self.sems is not None
    popped = nc._tile_sem_poison_stack.pop()
    assert popped is self._sem_poison
    nc.clear_and_free_semaphores(list(self.sems.allocated().values()))
    nc.all_engine_barrier()


tile.TileContext._drain_and_barrier = _patched_drain_and_barrier

_orig_to_json_bytes = bass.Bass.to_json_bytes
_MAX_WAITS = 1


def _to_json_split_waits(self):
    # walrus codegen caps sync waits per instruction (2-3 depending on the
    # struct); hoist excess on_wait entries onto same-engine NoOps placed
    # immediately before the instruction.
    import json as _json

    j = _json.loads(_orig_to_json_bytes(self))
    ctr = 0
    for f in j["functions"]:
        for blk in f["blocks"]:
            out = []
            for inst in blk["instructions"]:
                si = inst.get("sync_info") or {}
                waits = si.get("on_wait") or []
                if len(waits) > _MAX_WAITS:
                    for wt in waits[:-_MAX_WAITS]:
                        ctr += 1
                        out.append({
                            "debug": inst.get("debug", 0),
                            "engine": inst["engine"],
                            "ins": [], "outs": [],
                            "name": f"I-wfix-{ctr}",
                            "opcode": "NoOp",
                            "sync_info": {"on_update": [], "on_wait": [wt]},
                        })
                    si["on_wait"] = waits[-_MAX_WAITS:]
                out.append(inst)
            blk["instructions"] = out
    return _json.dumps(j).encode()


bass.Bass.to_json_bytes = _to_json_split_waits


def _mmdt(ap):
    return ap.bitcast(mybir.dt.float32r) if USE_F32R else ap


def build_kernel():
    f32 = mybir.dt.float32
    alu = mybir.AluOpType
    act = mybir.ActivationFunctionType

    nc = bass.Bass()
    bf16 = mybir.dt.bfloat16
    x_d = nc.dram_tensor("x", [C, H, W], f32, kind="ExternalInput")
    # bf16 weights + gray tensor: 1-pass matmuls (~216ns) instead of the
    # 3-pass fp32 HIGH mode (~630ns); adds ~3e-3 relative error against
    # the 2e-2 gate.
    w_d = nc.dram_tensor("w", [128, 9, Q], bf16, kind="ExternalInput")
    # normalized output leaves the chip as bf16 (host upcasts): the final
    # rounding adds ~2e-3 relative error against a 2e-2 gate and halves
    # the store traffic.
    y_d = nc.dram_tensor("y", [H, W], bf16, kind="ExternalOutput")

    a_rg = 0.3 / 0.59          # G = ((x0*a_rg + x1) * a_gb) + x2 = g/0.11
    a_gb = 0.59 / 0.11

    with tile.TileContext(nc) as tc:
        with (
            tc.tile_pool(name="persist", bufs=1) as persist,
            tc.tile_pool(name="xall", bufs=1) as xall,
            tc.tile_pool(name="gp", bufs=3) as gp,
            tc.tile_pool(name="sq", bufs=2) as sqp,
            tc.tile_pool(name="zb", bufs=9) as zbp,
            tc.tile_pool(name="gt", bufs=3) as gtp,
            tc.tile_pool(name="psum", bufs=4, space="PSUM") as psp,
            tc.tile_pool(name="psmall", bufs=2, space="PSUM") as pss,
        ):
            w_sb = persist.tile([128, 9, Q], bf16)
            nc.gpsimd.dma_start(out=w_sb[:], in_=w_d[:])

            # ---- issue ALL input loads up-front on SWDGE (gpsimd) ----
            # One 3-channel transfer per block: SWDGE stripes a single
            # transfer's descriptors across all 16 SDMA engines (by dest
            # partition), unlike HWDGE DIRECT2D which parks one transfer
            # on a single engine ring (~26 GB/s each).  Striping only
            # engages for 128-partition transfers, so EVERY block loads a
            # full 128-row window: block 0 loads rows [0,128), the tail
            # block loads rows [896,1024) (its weight variant selects the
            # last 17 rows; re-reading the overlap costs ~1.3 MB but keeps
            # all 16 engines in lockstep — a partial transfer pins to one
            # engine and makes it a straggler that delays every completion
            # semaphore behind it).
            xt = [None] * NBLK
            blk_geom = []
            for b in range(NBLK):
                r0 = Q * b
                q = min(Q, H - r0)            # 126, tail 16
                lo = min(max(r0 - 1, 0), H - 128)   # first loaded row
                blk_geom.append((r0, q, lo))
                t = xall.tile([128, C, W], f32, tag=f"x{b}", name=f"xt{b}")
                xt[b] = t
                # two column-half transfers per block: halves the per-engine
                # queue depth so completion semaphores fire sooner
                for cs in (0, 512):
                    nc.gpsimd.dma_start(
                        out=t[:, :, cs:cs + 512],
                        in_=x_d[:, lo:lo + 128, cs:cs + 512]
                        .rearrange("c r w -> r c w"))

            z_sb = persist.tile([128, NBLK, W], f32)
            # stats: [:, 0, :] partial row-sums, [:, 1, :] partial row-sumsq
            stat = persist.tile([128, 2, 2 * NBLK], f32)
            nc.vector.memset(stat[:], 0.0)

            ones_col = persist.tile([128, 1], f32)   # cross-partition reduce
            ones_row = persist.tile([1, 128], f32)   # partition broadcast
            nc.vector.memset(ones_col[:], 1.0)
            nc.vector.memset(ones_row[:], 1.0)

            red = persist.tile([128, 2], f32)
            t0 = persist.tile([1, 2], f32)
            t1 = persist.tile([1, 1], f32)
            var_t = persist.tile([1, 1], f32)
            s_t = persist.tile([1, 1], f32)
            ab1 = persist.tile([1, 2], f32)
            ab_sb = persist.tile([128, 2], f32)
            eps_t = persist.tile([1, 1], f32)
            nc.vector.memset(eps_t[:], EPS_EFF)
            # touch ACT early so its lazy activation-table load (~1.5us)
            # happens during the load phase, not inside block 0's chain
            warm_t = persist.tile([1, 1], f32)
            nc.scalar.activation(out=warm_t[:], in_=eps_t[:], func=act.Square)

            # ---- per-block: gray-convert, 3x2 conv matmuls, evict ----
            for b in range(NBLK):
                r0, q, lo = blk_geom[b]
                # weight variant: 0 interior, 3 top block, 6 tail block
                wj = 3 if b == 0 else (6 if b == NBLK - 1 else 0)

                # zero column padding at 0 and W+1 for the +-1 column
                # shifts of the conv matmuls
                g_t = gp.tile([128, W + 2], bf16)
                nc.gpsimd.memset(g_t[:, 0:1], 0.0)
                nc.gpsimd.memset(g_t[:, W + 1:W + 2], 0.0)
                gtmp = gtp.tile([128, W], f32, tag="gtmp", name=f"gtmp{b}")
                nc.vector.scalar_tensor_tensor(
                    out=gtmp[:, :], in0=xt[b][:, 0, :], scalar=a_rg,
                    in1=xt[b][:, 1, :], op0=alu.mult, op1=alu.add)
                nc.vector.scalar_tensor_tensor(
                    out=g_t[:, 1:W + 1], in0=gtmp[:, :], scalar=a_gb,
                    in1=xt[b][:, 2, :], op0=alu.mult, op1=alu.add)

                for h in range(2):
                    cs = 512 * h
                    ps = psp.tile([128, 512], f32)
                    for dji in range(3):  # dj = dji - 1; g_t col 1+cs+dj
                        nc.tensor.matmul(
                            ps[0:q, 0:512], w_sb[:, wj + dji, 0:q],
                            g_t[:, cs + dji:cs + dji + 512],
                            start=(dji == 0), stop=(dji == 2))

                    # evict + row sums and squares + row sumsq, both on ACT:
                    # DVE is the compute-phase bottleneck (gray STTs), ACT
                    # has headroom.
                    nc.scalar.activation(
                        out=z_sb[0:q, b, cs:cs + 512], in_=ps[0:q, 0:512],
                        func=act.Identity, bias=0.0, scale=1.0,
                        accum_out=stat[0:q, 0, 2 * b + h:2 * b + h + 1])
                    sq_t = sqp.tile([128, 512], f32)
                    nc.scalar.activation(
                        out=sq_t[0:q, :], in_=ps[0:q, 0:512], func=act.Square,
                        accum_out=stat[0:q, 1, 2 * b + h:2 * b + h + 1])

            # ---- stats finalize ----
            nc.vector.tensor_reduce(
                out=red[:, 0:2], in_=stat[:], axis=mybir.AxisListType.X,
                op=alu.add)
            pst = pss.tile([1, 2], f32)
            nc.tensor.matmul(pst[0:1, 0:2], ones_col[:, 0:1], red[:, 0:2],
                             start=True, stop=True)
            n_inv = 1.0 / float(H * W)
            nc.vector.tensor_scalar(out=t0[:], in0=pst[0:1, 0:2], scalar1=n_inv,
                                    scalar2=None, op0=alu.mult)
            nc.vector.tensor_tensor(out=t1[:], in0=t0[0:1, 0:1], in1=t0[0:1, 0:1],
                                    op=alu.mult)
            nc.vector.tensor_tensor(out=var_t[:], in0=t0[0:1, 1:2], in1=t1[:],
                                    op=alu.subtract)
            nc.scalar.activation(out=s_t[:], in_=var_t[:], func=act.Sqrt,
                                 bias=eps_t[0:1, 0:1], scale=1.0)
            nc.vector.reciprocal(ab1[0:1, 0:1], s_t[:])
            nc.vector.scalar_tensor_tensor(
                out=ab1[0:1, 1:2], in0=t0[0:1, 0:1], scalar=-1.0,
                in1=ab1[0:1, 0:1], op0=alu.mult, op1=alu.mult)
            psb = pss.tile([128, 2], f32)
            nc.tensor.matmul(psb[:, 0:2], ones_row[0:1, :], ab1[0:1, 0:2],
                             start=True, stop=True)
            nc.vector.tensor_copy(ab_sb[:], psb[:, 0:2])

            # ---- normalize to bf16 (mostly DVE) + store on both HWDGE rings ----
            for b in range(NBLK):
                r0, q, lo = blk_geom[b]
                zb = zbp.tile([128, W], bf16, tag="zb", name=f"zb{b}")
                if b % 3 == 1:
                    nc.scalar.activation(
                        out=zb[0:q, :], in_=z_sb[0:q, b, :],
                        func=act.Identity, bias=ab_sb[0:q, 1:2],
                        scale=ab_sb[0:q, 0:1])
                else:
                    nc.vector.tensor_scalar(
                        out=zb[0:q, :], in0=z_sb[0:q, b, :],
                        scalar1=ab_sb[0:q, 0:1], scalar2=ab_sb[0:q, 1:2],
                        op0=alu.mult, op1=alu.add)
                eng = nc.sync if b % 2 == 0 else nc.scalar
                eng.dma_start(out=y_d[r0:r0 + q, :], in_=zb[0:q, :])

    return nc


_NC = None


def kernel(x: np.ndarray) -> np.ndarray:
    global _NC
    from concourse.bass_utils import run_bass_kernel_spmd

    if _NC is None:
        _NC = build_kernel()
    import ml_dtypes

    w = _build_weights().astype(ml_dtypes.bfloat16)
    x = np.ascontiguousarray(x, dtype=np.float32)
    in_maps = [{"x": x[i], "w": w} for i in range(NCORES)]
    res = run_bass_kernel_spmd(_NC, in_maps, list(range(NCORES)))
    out = np.empty((B, C, H, W), np.float32)
    for i in range(NCORES):
        out[i] = np.asarray(res.results[i]["y"]).astype(np.float32)[None]
    return out


# revision 30
# speedup vs baseline: 1.0400x; 1.0400x over previous
"""LTPE kernel for Trainium2: RGB->gray, 8-neighbor weighted diff encoding,
instance norm, replicated to 3 channels.  Data-parallel over batch: one
sample per NeuronCore (8 cores).

Math: with g = 0.3 x0 + 0.59 x1 + 0.11 x2 and weights w_j = 2^j/255 at the
8 neighbor offsets, the reference output before the norm is
0.055*z + 0.5 where z = G - sum_j w_j * shift_j(G), G = g/0.11.
Instance norm is affine-invariant, so out = (z - mean_z) * rsqrt(var_z + EPS_EFF)
with EPS_EFF = 1e-5 / 0.055^2.

The 9-tap stencil is computed on the TensorEngine: for each column shift
dj in {-1,0,+1} a tridiagonal [128,126] lhsT applies all three row taps in
one matmul (PSUM-accumulated), on halo-free 126-row blocks.

v2: all input DMAs are issued up-front, alternating between the two HWDGE
rings (sync + scalar), so many transfers are in flight concurrently from
t=0 — a single in-flight HBM load drains at ~26 GB/s (one SDMA engine),
saturating ~358 GB/s needs 10+ concurrent transfers.  The normalize pass
is split between DVE and ACT and stores alternate rings too.
"""

import sys

sys.path.insert(0, "/opt/trn_rl_repo")

import numpy as np

import concourse.bass as bass
import concourse.mybir as mybir
import concourse.tile as tile
from concourse.vector_clock import ScopedClock

B, C, H, W = 8, 3, 1024, 1024
NCORES = 8
Q = 126              # output rows per block
NBLK = 9             # 8 full blocks + 1 tail block of 16 rows
EPS_EFF = 1e-5 / (0.5 * 0.11) ** 2

# neighbor offsets (di, dj) -> bit j;  kernel j weight = 2^j/255
OFFS = {(0, -1): 0, (1, -1): 1, (1, 0): 2, (1, 1): 3,
        (0, 1): 4, (-1, 1): 5, (-1, 0): 6, (-1, -1): 7}


def _tap(di, dj):
    v = 1.0 if (di == 0 and dj == 0) else 0.0
    if (di, dj) in OFFS:
        v -= 2.0 ** OFFS[(di, dj)] / 255.0
    return v


def _build_weights():
    # Interior blocks (cols 0-2): input partitions hold image rows r0-1+k,
    # so w[k, dji, m] = tap(k - m - 1, dj), nonzero at k = m, m+1, m+2.
    # Block 0 (cols 3-5): input starts at image row 0 (no row above), so
    # w[k, 3+dji, m] = tap(k - m, dj), nonzero at k = m-1, m, m+1.
    # Tail block (cols 6-8): input partitions hold rows 896+k (a full
    # 128-row window ending at the image bottom), output rows 1008+m for
    # m = 0..15, so w[k, 6+dji, m] = tap(k - 112 - m, dj), nonzero at
    # k = 111+m, 112+m, 113+m (k = 128 falls off the bottom = zero pad).
    w = np.zeros((128, 9, Q), np.float32)
    for dji, dj in enumerate((-1, 0, 1)):
        for m in range(Q):
            for k in (m, m + 1, m + 2):
                if k < 128:
                    w[k, dji, m] = _tap(k - m - 1, dj)
            for k in (m - 1, m, m + 1):
                if 0 <= k < 128:
                    w[k, 3 + dji, m] = _tap(k - m, dj)
        for m in range(H - Q * (NBLK - 1)):
            for k in (111 + m, 112 + m, 113 + m):
                if 0 <= k < 128:
                    w[k, 6 + dji, m] = _tap(k - 112 - m, dj)
    return w


def _patched_drain_and_barrier(self, tick_clock, wait_clock):
    # walrus rejects >1-2 sync waits on the kernel-tail Drain (CTRL
    # NO_STRUCT codegen); spread the global-clock waits one-per-nop.
    nc = self.nc
    carrier = nc.sync.nop()
    wait_clock.add_sem_waits(carrier.ins, ScopedClock({None: tick_clock.global_clock}))
    waits = list(carrier.ins.sync_info.on_wait or [])
    if len(waits) > 1:
        carrier.ins.sync_info.on_wait = waits[:1]
        for wt in waits[1:]:
            n = nc.sync.nop()
            n.ins.sync_info = mybir.SyncInfo(on_wait=[wt], on_update=[])
    nc.sync.drain()
    nc.all_engine_barrier()
    assert self.sems is not None
    popped = nc._tile_sem_poison_stack.pop()
    assert popped is self._sem_poison
    nc.clear_and_free_semaphores(list(self.sems.allocated().values()))
    nc.all_engine_barrier()


tile.TileContext._drain_and_barrier = _patched_drain_and_barrier

_orig_to_json_bytes = bass.Bass.to_json_bytes
_MAX_WAITS = 1


def _to_json_split_waits(self):
    # walrus codegen caps sync waits per instruction (2-3 depending on the
    # struct); hoist excess on_wait entries onto same-engine NoOps placed
    # immediately before the instruction.
    import json as _json

    j = _json.loads(_orig_to_json_bytes(self))
    ctr = 0
    for f in j["functions"]:
        for blk in f["blocks"]:
            out = []
            for inst in blk["instructions"]:
                si = inst.get("sync_info") or {}
                waits = si.get("on_wait") or []
                if len(waits) > _MAX_WAITS:
                    for wt in waits[:-_MAX_WAITS]:
                        ctr += 1
                        out.append({
                            "debug": inst.get("debug", 0),
                            "engine": inst["engine"],
                            "ins": [], "outs": [],
                            "name": f"I-wfix-{ctr}",
                            "opcode": "NoOp",
                            "sync_info": {"on_update": [], "on_wait": [wt]},
                        })
                    si["on_wait"] = waits[-_MAX_WAITS:]
                out.append(inst)
            blk["instructions"] = out
    return _json.dumps(j).encode()


bass.Bass.to_json_bytes = _to_json_split_waits


def build_kernel():
    f32 = mybir.dt.float32
    alu = mybir.AluOpType
    act = mybir.ActivationFunctionType

    nc = bass.Bass()
    bf16 = mybir.dt.bfloat16
    x_d = nc.dram_tensor("x", [C, H, W], f32, kind="ExternalInput")
    # bf16 weights + gray tensor: 1-pass matmuls (~216ns) instead of the
    # 3-pass fp32 HIGH mode (~630ns); adds ~3e-3 relative error against
    # the 2e-2 gate.
    w_d = nc.dram_tensor("w", [128, 9, Q], bf16, kind="ExternalInput")
    # normalized output leaves the chip as bf16 (host upcasts): the final
    # rounding adds ~2e-3 relative error against a 2e-2 gate and halves
    # the store traffic.
    y_d = nc.dram_tensor("y", [H, W], bf16, kind="ExternalOutput")

    a_rg = 0.3 / 0.59          # G = ((x0*a_rg + x1) * a_gb) + x2 = g/0.11
    a_gb = 0.59 / 0.11

    with tile.TileContext(nc) as tc:
        with (
            tc.tile_pool(name="persist", bufs=1) as persist,
            tc.tile_pool(name="xall", bufs=1) as xall,
            tc.tile_pool(name="gp", bufs=3) as gp,
            tc.tile_pool(name="sq", bufs=2) as sqp,
            tc.tile_pool(name="zb", bufs=9) as zbp,
            tc.tile_pool(name="gt", bufs=3) as gtp,
            tc.tile_pool(name="psum", bufs=4, space="PSUM") as psp,
            tc.tile_pool(name="psmall", bufs=2, space="PSUM") as pss,
        ):
            w_sb = persist.tile([128, 9, Q], bf16)
            nc.gpsimd.dma_start(out=w_sb[:], in_=w_d[:])

            # ---- issue ALL input loads up-front on SWDGE (gpsimd) ----
            # One 3-channel transfer per block: SWDGE stripes a single
            # transfer's descriptors across all 16 SDMA engines (by dest
            # partition), unlike HWDGE DIRECT2D which parks one transfer
            # on a single engine ring (~26 GB/s each).  Striping only
            # engages for 128-partition transfers, so EVERY block loads a
            # full 128-row window: block 0 loads rows [0,128), the tail
            # block loads rows [896,1024) (its weight variant selects the
            # last 17 rows; re-reading the overlap costs ~1.3 MB but keeps
            # all 16 engines in lockstep — a partial transfer pins to one
            # engine and makes it a straggler that delays every completion
            # semaphore behind it).
            xt = [None] * NBLK
            blk_geom = []
            for b in range(NBLK):
                r0 = Q * b
                q = min(Q, H - r0)            # 126, tail 16
                lo = min(max(r0 - 1, 0), H - 128)   # first loaded row
                blk_geom.append((r0, q, lo))
                t = xall.tile([128, C, W], f32, tag=f"x{b}", name=f"xt{b}")
                xt[b] = t
                nc.gpsimd.dma_start(
                    out=t[:, :, :],
                    in_=x_d[:, lo:lo + 128, :].rearrange("c r w -> r c w"))

            z_sb = persist.tile([128, NBLK, W], f32)
            # stats: [:, 0, :] partial row-sums, [:, 1, :] partial row-sumsq
            stat = persist.tile([128, 2, 2 * NBLK], f32)
            nc.vector.memset(stat[:], 0.0)

            ones_col = persist.tile([128, 1], f32)   # cross-partition reduce
            ones_row = persist.tile([1, 128], f32)   # partition broadcast
            nc.vector.memset(ones_col[:], 1.0)
            nc.vector.memset(ones_row[:], 1.0)

            red = persist.tile([128, 2], f32)
            t0 = persist.tile([1, 2], f32)
            t1 = persist.tile([1, 1], f32)
            var_t = persist.tile([1, 1], f32)
            s_t = persist.tile([1, 1], f32)
            ab1 = persist.tile([1, 2], f32)
            ab_sb = persist.tile([128, 2], f32)
            eps_t = persist.tile([1, 1], f32)
            nc.vector.memset(eps_t[:], EPS_EFF)
            # touch ACT early so its lazy activation-table load (~1.5us)
            # happens during the load phase, not inside block 0's chain
            warm_t = persist.tile([1, 1], f32)
            nc.scalar.activation(out=warm_t[:], in_=eps_t[:], func=act.Square)

            # ---- per-block: gray-convert, 3x2 conv matmuls, evict ----
            for b in range(NBLK):
                r0, q, lo = blk_geom[b]
                # weight variant: 0 interior, 3 top block, 6 tail block
                wj = 3 if b == 0 else (6 if b == NBLK - 1 else 0)

                # zero column padding at 0 and W+1 for the +-1 column
                # shifts of the conv matmuls
                g_t = gp.tile([128, W + 2], bf16)
                nc.gpsimd.memset(g_t[:, 0:1], 0.0)
                nc.gpsimd.memset(g_t[:, W + 1:W + 2], 0.0)
                gtmp = gtp.tile([128, W], f32, tag="gtmp", name=f"gtmp{b}")
                nc.vector.scalar_tensor_tensor(
                    out=gtmp[:, :], in0=xt[b][:, 0, :], scalar=a_rg,
                    in1=xt[b][:, 1, :], op0=alu.mult, op1=alu.add)
                nc.vector.scalar_tensor_tensor(
                    out=g_t[:, 1:W + 1], in0=gtmp[:, :], scalar=a_gb,
                    in1=xt[b][:, 2, :], op0=alu.mult, op1=alu.add)

                for h in range(2):
                    cs = 512 * h
                    ps = psp.tile([128, 512], f32)
                    for dji in range(3):  # dj = dji - 1; g_t col 1+cs+dj
                        nc.tensor.matmul(
                            ps[0:q, 0:512], w_sb[:, wj + dji, 0:q],
                            g_t[:, cs + dji:cs + dji + 512],
                            start=(dji == 0), stop=(dji == 2))

                    # evict + row sums and squares + row sumsq, both on ACT:
                    # DVE is the compute-phase bottleneck (gray STTs), ACT
                    # has headroom.
                    nc.scalar.activation(
                        out=z_sb[0:q, b, cs:cs + 512], in_=ps[0:q, 0:512],
                        func=act.Identity, bias=0.0, scale=1.0,
                        accum_out=stat[0:q, 0, 2 * b + h:2 * b + h + 1])
                    sq_t = sqp.tile([128, 512], f32)
                    nc.scalar.activation(
                        out=sq_t[0:q, :], in_=ps[0:q, 0:512], func=act.Square,
                        accum_out=stat[0:q, 1, 2 * b + h:2 * b + h + 1])

            # ---- stats finalize ----
            nc.vector.tensor_reduce(
                out=red[:, 0:2], in_=stat[:], axis=mybir.AxisListType.X,
                op=alu.add)
            pst = pss.tile([1, 2], f32)
            nc.tensor.matmul(pst[0:1, 0:2], ones_col[:, 0:1], red[:, 0:2],
                             start=True, stop=True)
            n_inv = 1.0 / float(H * W)
            nc.vector.tensor_scalar(out=t0[:], in0=pst[0:1, 0:2], scalar1=n_inv,
                                    scalar2=None, op0=alu.mult)
            nc.vector.tensor_tensor(out=t1[:], in0=t0[0:1, 0:1], in1=t0[0:1, 0:1],
                                    op=alu.mult)
            nc.vector.tensor_tensor(out=var_t[:], in0=t0[0:1, 1:2], in1=t1[:],
                                    op=alu.subtract)
            nc.scalar.activation(out=s_t[:], in_=var_t[:], func=act.Sqrt,
                                 bias=eps_t[0:1, 0:1], scale=1.0)
            nc.vector.reciprocal(ab1[0:1, 0:1], s_t[:])
            nc.vector.scalar_tensor_tensor(
                out=ab1[0:1, 1:2], in0=t0[0:1, 0:1], scalar=-1.0,
                in1=ab1[0:1, 0:1], op0=alu.mult, op1=alu.mult)
            psb = pss.tile([128, 2], f32)
            nc.tensor.matmul(psb[:, 0:2], ones_row[0:1, :], ab1[0:1, 0:2],
                             start=True, stop=True)
            nc.vector.tensor_copy(ab_sb[:], psb[:, 0:2])

            # ---- normalize to bf16 (mostly DVE) + store on both HWDGE rings ----
            for b in range(NBLK):
                r0, q, lo = blk_geom[b]
                zb = zbp.tile([128, W], bf16, tag="zb", name=f"zb{b}")
                if b % 3 == 1:
                    nc.scalar.activation(
                        out=zb[0:q, :], in_=z_sb[0:q, b, :],
                        func=act.Identity, bias=ab_sb[0:q, 1:2],
                        scale=ab_sb[0:q, 0:1])
                else:
                    nc.vector.tensor_scalar(
                        out=zb[0:q, :], in0=z_sb[0:q, b, :],
                        scalar1=ab_sb[0:q, 0:1], scalar2=ab_sb[0:q, 1:2],
                        op0=alu.mult, op1=alu.add)
                eng = nc.sync if b % 2 == 0 else nc.scalar
                eng.dma_start(out=y_d[r0:r0 + q, :], in_=zb[0:q, :])

    return nc


_NC = None


def kernel(x: np.ndarray) -> np.ndarray:
    global _NC
    from concourse.bass_utils import run_bass_kernel_spmd

    if _NC is None:
        _NC = build_kernel()
    import ml_dtypes

    w = _build_weights().astype(ml_dtypes.bfloat16)
    x = np.ascontiguousarray(x, dtype=np.float32)
    in_maps = [{"x": x[i], "w": w} for i in range(NCORES)]
    res = run_bass_kernel_spmd(_NC, in_maps, list(range(NCORES)))
    out = np.empty((B, C, H, W), np.float32)
    for i in range(NCORES):
        out[i] = np.asarray(res.results[i]["y"]).astype(np.float32)[None]
    return out
